# revision 1
# baseline (speedup 1.0000x reference)
"""Trainium2 Bass kernel for nn_Attention_15899968929956.

Block-diagonal GNN message passing == dense per-system attention:
64 systems x 64 electrons, DIM=256, 8 heads x head_dim 32. Edges are all
intra-system pairs, so per (system, head):
  S'[j, i] = K[j] . Q[i] / sqrt(hd)           (j, i in [0, 64))
  P[j, i]  = exp(S') / sum_i' exp(S'[j, i'])  (softmax segmented by key j)
  attn[i]  = sum_j P[j, i] * V[j]
then out = LN2(h3 + silu(h3 @ W_mlp + b)), h3 = LN1(h + attn @ W_out).

Sharding: 8 systems (512 electrons) per NeuronCore, parameters replicated.

Layouts per core (all SBUF tiles 128 partitions):
  hsb  [128, 4, 256]   natural rows (block n = device rows 128n..128n+128)
  hT   [128, 2, 512]   h transposed (chunk c = features 128c.., col = row idx)
  QT/KT[128, 2, 512]   transposed Q/K (chunk c = heads 4c..4c+3, 32 rows each)
  Vn   [128, 4, 256]   V natural, pair q = rows 128q..
  aT   [128, 2, 512]   attn transposed (chunk c = heads 4c.., col = row idx)
Small matmuls use PE array tiling: scores K=32/M=64 (8 concurrent tiles),
PV K=64/M=32 (8 concurrent tiles). Big matmuls use float32r (full PE rate).
"""

import sys

if "/opt/trn_rl_repo" not in sys.path:
    sys.path.insert(0, "/opt/trn_rl_repo")

from contextlib import ExitStack

import numpy as np

N_SYS = 64
N_ELEC = 64
DIM = 256
HEADS = 8
HD = DIM // HEADS  # 32
EPS = 1e-6
NCORES = 8
SPC = N_SYS // NCORES      # systems per core = 8
R = SPC * N_ELEC           # rows per core = 512
NPAIR = SPC // 2           # system pairs per core = 4
NBLK = R // 128            # 128-row blocks per core = 4
SCALE = 1.0 / float(np.sqrt(HD))

# "f32" (exact) or "f32r" (reduced-precision multiplies on the big matmuls,
# full PE rate at N>=256 instead of 1/4 rate for fp32)
BIG_MM_DTYPE = "f32r"

_BUILD_CACHE: dict = {}


def _expected_edges():
    ii, jj = np.meshgrid(np.arange(N_ELEC), np.arange(N_ELEC), indexing="ij")
    offs = (np.arange(N_SYS) * N_ELEC)[:, None, None]
    ei = (offs + ii[None]).reshape(-1).astype(np.int32)
    ej = (offs + jj[None]).reshape(-1).astype(np.int32)
    return ei, ej


def _edges_are_blockdense(e_e_i, e_e_j):
    ei, ej = _expected_edges()
    a = np.asarray(e_e_i).ravel()
    b = np.asarray(e_e_j).ravel()
    if a.shape != ei.shape or b.shape != ej.shape:
        return False
    if np.array_equal(a, ei) and np.array_equal(b, ej):
        return True
    key = a.astype(np.int64) * (N_SYS * N_ELEC) + b.astype(np.int64)
    kref = ei.astype(np.int64) * (N_SYS * N_ELEC) + ej.astype(np.int64)
    return np.array_equal(np.sort(key), np.sort(kref))


def _reference_np(h_one, W_qkv, W_out, ln1_scale, ln1_bias, W_mlp, b_mlp,
                  ln2_scale, ln2_bias, e_e_i, e_e_j):
    """Numpy fallback for arbitrary edge lists (never hit for the real inputs)."""
    h = np.asarray(h_one, np.float64)
    n = h.shape[0]
    qkv = h @ np.asarray(W_qkv, np.float64)
    Q, K, V = np.split(qkv, 3, axis=-1)
    Q = Q.reshape(n, HEADS, HD)
    K = K.reshape(n, HEADS, HD)
    V = V.reshape(n, HEADS, HD)
    ei = np.asarray(e_e_i).ravel()
    ej = np.asarray(e_e_j).ravel()
    A = np.einsum("ehd,ehd->eh", Q[ei], K[ej]) / np.sqrt(HD)
    mx = np.full((n, HEADS), -np.inf)
    np.maximum.at(mx, ej, A)
    e = np.exp(A - mx[ej])
    den = np.zeros((n, HEADS))
    np.add.at(den, ej, e)
    P = e / den[ej]
    attn = np.zeros((n, HEADS, HD))
    np.add.at(attn, ei, P[..., None] * V[ej])
    attn = attn.reshape(n, DIM)
    hh = h + attn @ np.asarray(W_out, np.float64)

    def ln(x, s, b):
        mu = x.mean(-1, keepdims=True)
        var = ((x - mu) ** 2).mean(-1, keepdims=True)
        return (x - mu) / np.sqrt(var + EPS) * np.asarray(s, np.float64) \
            + np.asarray(b, np.float64)

    hh = ln(hh, ln1_scale, ln1_bias)
    m = hh @ np.asarray(W_mlp, np.float64) + np.asarray(b_mlp, np.float64)
    hh = hh + m / (1.0 + np.exp(-m))
    hh = ln(hh, ln2_scale, ln2_bias)
    return hh.astype(np.float32)


def _build(flags, chain=1):
    """Build + compile the Bass program.

    flags = (ln1_aff, ln2_aff, mlp_bias, big_dt). chain>1 repeats the whole
    body, iteration t reading h from the out tensor written by t-1 (timing
    harness: marginal iteration == steady-state kernel incl. all DMA).
    """
    key = (flags, chain)
    if key in _BUILD_CACHE:
        return _BUILD_CACHE[key]

    import concourse.bass as bass
    import concourse.mybir as mybir
    import concourse.tile as tile
    from concourse import bacc
    from concourse.masks import make_identity

    ln1_aff, ln2_aff, mlp_bias, big_dt = flags
    f32 = mybir.dt.float32
    mdt = mybir.dt.float32r if big_dt == "f32r" else mybir.dt.float32
    PS = bass.MemorySpace.PSUM

    nc = bacc.Bacc("TRN2", target_bir_lowering=False, debug=False,
                   num_devices=NCORES)

    h_d = nc.dram_tensor("h", [R, DIM], f32, kind="ExternalInput")
    wq_d = nc.dram_tensor("wq", [DIM, 3 * DIM], mdt, kind="ExternalInput")
    wo_d = nc.dram_tensor("wo", [DIM, DIM], mdt, kind="ExternalInput")
    wm_d = nc.dram_tensor("wm", [DIM, DIM], mdt, kind="ExternalInput")
    if ln1_aff:
        ln1s_d = nc.dram_tensor("ln1s", [DIM], f32, kind="ExternalInput")
        ln1b_d = nc.dram_tensor("ln1b", [DIM], f32, kind="ExternalInput")
    if ln2_aff:
        ln2s_d = nc.dram_tensor("ln2s", [DIM], f32, kind="ExternalInput")
        ln2b_d = nc.dram_tensor("ln2b", [DIM], f32, kind="ExternalInput")
    if mlp_bias:
        bm_d = nc.dram_tensor("bm", [DIM], f32, kind="ExternalInput")
    out_d = nc.dram_tensor("out", [R, DIM], f32, kind="ExternalOutput")

    Exp = mybir.ActivationFunctionType.Exp
    Silu = mybir.ActivationFunctionType.Silu
    Sqrt = mybir.ActivationFunctionType.Sqrt
    SUB = mybir.AluOpType.subtract
    MUL = mybir.AluOpType.mult
    X = mybir.AxisListType.X

    with tile.TileContext(nc) as tc:
        with (
            tc.tile_pool(name="per", bufs=1) as per,    # persistent sbuf
            tc.tile_pool(name="rot", bufs=3) as rot,    # rotating sbuf
            tc.tile_pool(name="rot3", bufs=3) as rot3,
            tc.tile_pool(name="rot4", bufs=4) as rot4,
            tc.tile_pool(name="small", bufs=4) as small,
        ):
            # ---- persistent SBUF ----
            ident = per.tile([128, 128], f32, tag="ident")
            make_identity(nc, ident)
            epst = per.tile([128, 1], f32, tag="epst")
            nc.vector.memset(epst, EPS)
            zt = per.tile([128, 1], f32, tag="zt")
            nc.vector.memset(zt, 0.0)
            wq = per.tile([128, 2, 3 * DIM], mdt, tag="wq")
            wo = per.tile([128, 2, DIM], mdt, tag="wo")
            wm = per.tile([128, 2, DIM], mdt, tag="wm")
            hsb = per.tile([128, NBLK, DIM], f32, tag="hsb")
            hT = per.tile([128, 2, R], mdt, tag="hT")
            QT = per.tile([128, 2, R], f32, tag="QT")
            KT = per.tile([128, 2, R], f32, tag="KT")
            Vn = per.tile([128, NPAIR, DIM], f32, tag="Vn")
            aT = per.tile([128, 2, R], mdt, tag="aT")  # attnT sbuf
            if ln1_aff:
                ln1s = per.tile([128, DIM], f32, tag="ln1s")
                ln1b = per.tile([128, DIM], f32, tag="ln1b")
            if ln2_aff:
                ln2s = per.tile([128, DIM], f32, tag="ln2s")
                ln2b = per.tile([128, DIM], f32, tag="ln2b")
            if mlp_bias:
                bm = per.tile([128, DIM], f32, tag="bm")

            for it in range(chain):
                h_src = h_d if it == 0 else out_d

                for n in range(NBLK):
                    nc.sync.dma_start(out=hsb[:, n, :],
                                      in_=h_src[128 * n:128 * (n + 1), :])
                for k in range(2):
                    nc.sync.dma_start(out=wq[:, k, :],
                                      in_=wq_d[128 * k:128 * (k + 1), :])
                nc.sync.dma_start(
                    out=wo, in_=wo_d[:].rearrange("(c p) n -> p c n", p=128))
                nc.sync.dma_start(
                    out=wm, in_=wm_d[:].rearrange("(c p) n -> p c n", p=128))
                if ln1_aff:
                    nc.sync.dma_start(out=ln1s, in_=ln1s_d[:].to_broadcast([128, DIM]))
                    nc.sync.dma_start(out=ln1b, in_=ln1b_d[:].to_broadcast([128, DIM]))
                if ln2_aff:
                    nc.sync.dma_start(out=ln2s, in_=ln2s_d[:].to_broadcast([128, DIM]))
                    nc.sync.dma_start(out=ln2b, in_=ln2b_d[:].to_broadcast([128, DIM]))
                if mlp_bias:
                    nc.sync.dma_start(out=bm, in_=bm_d[:].to_broadcast([128, DIM]))

                # ---- phase A: h -> hT transposes, QKV projections ----
                pa = ExitStack()
                pst = pa.enter_context(
                    tc.tile_pool(name=f"pst{it}", bufs=3, space=PS))
                psqk = pa.enter_context(
                    tc.tile_pool(name=f"psqk{it}", bufs=3, space=PS))
                psv = pa.enter_context(
                    tc.tile_pool(name=f"psv{it}", bufs=2, space=PS))
                for half in range(2):
                    for c in range(2):
                        tp = pst.tile([128, 256], f32, tag="tp")
                        for k in range(2):
                            n = 2 * half + k
                            nc.tensor.transpose(
                                tp[:, 128 * k:128 * (k + 1)],
                                hsb[:, n, 128 * c:128 * (c + 1)], ident)
                        if c == 0:
                            nc.vector.tensor_copy(
                                out=hT[:, c, 256 * half:256 * (half + 1)], in_=tp)
                        else:
                            nc.scalar.copy(
                                out=hT[:, c, 256 * half:256 * (half + 1)], in_=tp)

                # qkvT: feature chunks t: 0,1 -> QT; 2,3 -> KT.
                # Half-width (N=256) pieces, halves-first, with that half's
                # V pairs interleaved, so pair-group 0's attention inputs
                # (QT/KT half 0 + Vn pairs 0,1) are ready earliest.
                for half in range(2):
                    cols = slice(256 * half, 256 * (half + 1))
                    for t in range(4):
                        ps = psqk.tile([128, 256], f32, tag="psqk")
                        for k in range(2):
                            nc.tensor.matmul(
                                ps,
                                wq[:, k, 128 * t:128 * (t + 1)],
                                hT[:, k, cols],
                                start=(k == 0), stop=(k == 1),
                            )
                        dst = QT if t < 2 else KT
                        nc.vector.tensor_copy(out=dst[:, t % 2, cols], in_=ps)
                    for q in (2 * half, 2 * half + 1):
                        ps = psv.tile([128, DIM], f32, tag="psv")
                        for k in range(2):
                            nc.tensor.matmul(
                                ps,
                                hT[:, k, 128 * q:128 * (q + 1)],
                                wq[:, k, 2 * DIM:3 * DIM],
                                start=(k == 0), stop=(k == 1),
                            )
                        nc.scalar.copy(out=Vn[:, q, :], in_=ps)

                pa.close()
                # ---- phase B: attention ----
                pb = ExitStack()
                pat = pb.enter_context(
                    tc.tile_pool(name=f"pat{it}", bufs=1, space=PS))
                psS = pb.enter_context(
                    tc.tile_pool(name=f"psS{it}", bufs=4, space=PS))
                at_ps = [[pat.tile([128, NPAIR * 64], f32, tag=f"at{c}{p}",
                                   name=f"at_ps{c}{p}")
                          for p in range(2)] for c in range(2)]

                def emit_scores(g):
                    sp = [psS.tile([128, 256], f32, tag="sp",
                                   name=f"sp{g}_{b}") for b in range(4)]
                    for p2 in range(2):
                        q = 2 * g + p2
                        for ch in range(2):
                            for hh in range(4):
                                for par in range(2):
                                    col = 64 * (2 * q + par)
                                    nc.tensor.matmul(
                                        sp[hh][64 * par:64 * (par + 1),
                                               128 * p2 + 64 * ch:
                                               128 * p2 + 64 * (ch + 1)],
                                        KT[:, ch, :][32 * hh:32 * (hh + 1),
                                                     col:col + 64],
                                        QT[:, ch, :][32 * hh:32 * (hh + 1),
                                                     col:col + 64],
                                        tile_position=(32 * hh, 64 * par),
                                        start=True, stop=True,
                                    )
                    return sp

                def emit_softmax(g, sp):
                    # exp (scaled), one wide op per bank; E cols:
                    # 512*p2 + 64*head + i (head-major within pair)
                    E = rot.tile([128, 2 * 512], f32, tag="E",
                                 name=f"E{g}")
                    Eg = E[:].rearrange("p (s c h i) -> p s c h i",
                                        s=2, c=2, i=64)
                    for hh in range(4):
                        nc.scalar.activation(
                            out=Eg[:, :, :, hh, :],
                            in_=sp[hh][:].rearrange("p (s c i) -> p s c i",
                                                    s=2, i=64),
                            func=Exp, bias=zt, scale=SCALE,
                        )
                    Dn = small.tile([128, 16], f32, tag="Dn", name=f"Dn{g}")
                    nc.vector.reduce_sum(
                        out=Dn, in_=E[:].rearrange("p (m i) -> p m i", i=64),
                        axis=X)
                    Rc = small.tile([128, 16], f32, tag="Rc", name=f"Rc{g}")
                    nc.vector.reciprocal(out=Rc, in_=Dn)
                    return E, Rc

                def emit_pv(g, E, Rc):
                    for p2 in range(2):
                        q = 2 * g + p2
                        # V' = V * (1/D), broadcast per head (gpsimd: sbuf)
                        Vp = rot.tile([128, DIM], f32, tag="Vp",
                                      name=f"Vp{g}_{p2}")
                        nc.gpsimd.tensor_mul(
                            Vp[:].rearrange("p (h d) -> p h d", d=HD),
                            Vn[:, q, :].rearrange("p (h d) -> p h d", d=HD),
                            Rc[:, 8 * p2:8 * (p2 + 1)].to_broadcast(
                                [128, 8, HD]),
                        )
                        # attn^T[d, i] = sum_j V'[j, d] E[j, i]
                        for ch in range(2):
                            for hh in range(4):
                                hg = 4 * ch + hh
                                for par in range(2):
                                    nc.tensor.matmul(
                                        at_ps[ch][par][32 * hh:32 * (hh + 1),
                                                       64 * q:64 * (q + 1)],
                                        Vp[64 * par:64 * (par + 1),
                                           32 * hg:32 * (hg + 1)],
                                        E[64 * par:64 * (par + 1),
                                          512 * p2 + 64 * hg:
                                          512 * p2 + 64 * (hg + 1)],
                                        tile_position=(64 * par, 32 * hh),
                                        start=True, stop=True,
                                    )

                # interleave the two pair-groups: group 1's scores run on PE
                # while group 0's softmax completes on ACT/DVE/Pool
                sp0 = emit_scores(0)
                E0, Rc0 = emit_softmax(0, sp0)
                sp1 = emit_scores(1)
                emit_pv(0, E0, Rc0)
                E1, Rc1 = emit_softmax(1, sp1)
                emit_pv(1, E1, Rc1)

                # attnT psum -> sbuf (interleave parities into device order)
                for c in range(2):
                    av = aT[:, c, :].rearrange("p (q s e) -> p q s e", s=2, e=64)
                    nc.vector.tensor_copy(
                        out=av[:, :, 0, :],
                        in_=at_ps[c][0][:].rearrange("p (q e) -> p q e", e=64))
                    nc.scalar.copy(
                        out=av[:, :, 1, :],
                        in_=at_ps[c][1][:].rearrange("p (q e) -> p q e", e=64))

                pb.close()
                # ---- phase C: W_out, residual, LN1, MLP, LN2, store ----
                # Split into sub-phases so same-table ACT ops cluster:
                # C1 all sqrts (LN1), C2 all silus, C3 all sqrts (LN2).
                pc = ExitStack()
                psh2 = pc.enter_context(
                    tc.tile_pool(name=f"psh2{it}", bufs=3, space=PS))
                psm = pc.enter_context(
                    tc.tile_pool(name=f"psm{it}", bufs=2, space=PS))
                pst = pc.enter_context(
                    tc.tile_pool(name=f"pst2{it}", bufs=3, space=PS))
                r1s, h3s, mvs, rss, h4s, mv2s, rs2s = [], [], [], [], [], [], []
                # C1: W_out matmul + residual + LN1 stats + rstd
                for n in range(NBLK):
                    ps2 = psh2.tile([128, DIM], f32, tag="ps2")
                    for c in range(2):
                        nc.tensor.matmul(
                            ps2,
                            aT[:, c, 128 * n:128 * (n + 1)],
                            wo[:, c, :],
                            start=(c == 0), stop=(c == 1),
                        )
                    r1 = rot4.tile([128, DIM], f32, tag="r1")
                    nc.vector.tensor_add(r1, hsb[:, n, :], ps2)
                    st = small.tile([128, 6], f32, tag="st")
                    nc.vector.bn_stats(out=st, in_=r1)
                    mv = small.tile([128, 2], f32, tag="mv")
                    nc.vector.bn_aggr(out=mv, in_=st)
                    sd = small.tile([128, 1], f32, tag="sd")
                    nc.scalar.activation(out=sd, in_=mv[:, 1:2], func=Sqrt, bias=epst)
                    rs = small.tile([128, 1], f32, tag="rs")
                    nc.vector.reciprocal(out=rs, in_=sd)
                    r1s.append(r1); mvs.append(mv); rss.append(rs)
                # C2: normalize + transpose + MLP + silu + residual2
                for n in range(NBLK):
                    h3 = rot4.tile([128, DIM], f32, tag="h3")
                    nc.gpsimd.tensor_scalar(h3, r1s[n], mvs[n][:, 0:1], rss[n],
                                            op0=SUB, op1=MUL)
                    if ln1_aff:
                        nc.vector.tensor_mul(h3, h3, ln1s)
                        nc.vector.tensor_add(h3, h3, ln1b)
                    h3t = rot.tile([128, 2, 128], mdt, tag="h3t")
                    tp = pst.tile([128, 256], f32, tag="tp")
                    for c in range(2):
                        nc.tensor.transpose(
                            tp[:, 128 * c:128 * (c + 1)],
                            h3[:, 128 * c:128 * (c + 1)], ident)
                    if n % 2 == 0:
                        nc.vector.tensor_copy(
                            out=h3t[:].rearrange("p c x -> p (c x)"), in_=tp)
                    else:
                        nc.scalar.copy(
                            out=h3t[:].rearrange("p c x -> p (c x)"), in_=tp)
                    psm_t = psm.tile([128, DIM], f32, tag="psm")
                    for c in range(2):
                        nc.tensor.matmul(
                            psm_t,
                            h3t[:, c, :],
                            wm[:, c, :],
                            start=(c == 0), stop=(c == 1),
                        )
                    if mlp_bias:
                        nc.vector.tensor_add(psm_t, psm_t, bm)
                    sl = rot.tile([128, DIM], f32, tag="sl")
                    nc.scalar.activation(out=sl, in_=psm_t, func=Silu, bias=zt)
                    h4 = rot4.tile([128, DIM], f32, tag="h4")
                    nc.gpsimd.tensor_add(h4, h3, sl)
                    h4s.append(h4)
                # C3: LN2 + store
                for n in range(NBLK):
                    st2 = small.tile([128, 6], f32, tag="st2")
                    nc.vector.bn_stats(out=st2, in_=h4s[n])
                    mv2 = small.tile([128, 2], f32, tag="mv2")
                    nc.vector.bn_aggr(out=mv2, in_=st2)
                    sd2 = small.tile([128, 1], f32, tag="sd2")
                    nc.scalar.activation(out=sd2, in_=mv2[:, 1:2], func=Sqrt,
                                         bias=epst)
                    rs2 = small.tile([128, 1], f32, tag="rs2")
                    nc.vector.reciprocal(out=rs2, in_=sd2)
                    ot = rot3.tile([128, DIM], f32, tag="ot")
                    nc.gpsimd.tensor_scalar(ot, h4s[n], mv2[:, 0:1], rs2,
                                            op0=SUB, op1=MUL)
                    if ln2_aff:
                        nc.vector.tensor_mul(ot, ot, ln2s)
                        nc.vector.tensor_add(ot, ot, ln2b)
                    nc.sync.dma_start(out=out_d[128 * n:128 * (n + 1), :], in_=ot)

                pc.close()

    nc.compile()
    _BUILD_CACHE[key] = nc
    return nc


def kernel(h_one, W_qkv, W_out, ln1_scale, ln1_bias, W_mlp, b_mlp,
           ln2_scale, ln2_bias, e_e_i, e_e_j, _trace=False, _chain=1):
    h_one = np.ascontiguousarray(np.asarray(h_one, np.float32))
    W_qkv = np.ascontiguousarray(np.asarray(W_qkv, np.float32))
    W_out = np.ascontiguousarray(np.asarray(W_out, np.float32))
    W_mlp = np.ascontiguousarray(np.asarray(W_mlp, np.float32))
    ln1_scale = np.asarray(ln1_scale, np.float32)
    ln1_bias = np.asarray(ln1_bias, np.float32)
    ln2_scale = np.asarray(ln2_scale, np.float32)
    ln2_bias = np.asarray(ln2_bias, np.float32)
    b_mlp = np.asarray(b_mlp, np.float32)

    if not _edges_are_blockdense(e_e_i, e_e_j):
        return _reference_np(h_one, W_qkv, W_out, ln1_scale, ln1_bias, W_mlp,
                             b_mlp, ln2_scale, ln2_bias, e_e_i, e_e_j)

    ln1_aff = not (np.all(ln1_scale == 1.0) and np.all(ln1_bias == 0.0))
    ln2_aff = not (np.all(ln2_scale == 1.0) and np.all(ln2_bias == 0.0))
    mlp_bias = not np.all(b_mlp == 0.0)
    nc = _build((ln1_aff, ln2_aff, mlp_bias, BIG_MM_DTYPE), chain=_chain)

    from concourse.bass_utils import run_bass_kernel_spmd

    in_maps = []
    for c in range(NCORES):
        m = {
            "h": h_one[R * c:R * (c + 1)],
            "wq": W_qkv,
            "wo": W_out,
            "wm": W_mlp,
        }
        if ln1_aff:
            m["ln1s"] = ln1_scale
            m["ln1b"] = ln1_bias
        if ln2_aff:
            m["ln2s"] = ln2_scale
            m["ln2b"] = ln2_bias
        if mlp_bias:
            m["bm"] = b_mlp
        in_maps.append(m)

    try:
        res = run_bass_kernel_spmd(nc, in_maps, core_ids=list(range(NCORES)),
                                   trace=_trace)
    except ModuleNotFoundError:
        # NTFF trace hook unavailable under this axon client
        res = run_bass_kernel_spmd(nc, in_maps, core_ids=list(range(NCORES)),
                                   trace=False)
    out = np.concatenate([res.results[c]["out"] for c in range(NCORES)], axis=0)
    if _trace:
        kernel._last_results = res
    return out



# revision 8
# speedup vs baseline: 5.4688x; 5.4688x over previous
"""Trainium2 Bass kernel v2 for nn_Attention_15899968929956.

Block-diagonal GNN message passing == dense per-system attention:
64 systems x 64 electrons, DIM=256, 8 heads x head_dim 32.
  S'[j, i] = K[j] . Q[i] / sqrt(hd)            (softmax segmented by key j)
  attn[i]  = sum_j P[j, i] * V[j]
  h3 = LN1(h + attn @ W_out); out = LN2(h3 + silu(h3 @ W_mlp))

v2 design (vs v1):
- all matmuls bf16 (fp32 runs the PE at 1/4 rate for N<256; bf16 is full
  rate at any N, critical for the 128 N=64 attention matmuls)
- weights pre-cast to bf16 on host; h and out bf16 in DRAM (host casts the
  gathered output back to fp32; tolerance is 2e-2)
- hT loaded via DMA-XBAR transpose straight from DRAM (no PE transposes,
  no psum->sbuf copies for hT)
- MLP folded: r1 @ W_mlp = h @ W_mlp + attn @ (W_out @ W_mlp), with the
  LN1 mean handled as a rank-1 update  - mu (x) colsum(W_mlp)  via a K=1
  matmul, and rstd applied inside the silu-exp activation scale. This
  removes the h3 transpose + second big-matmul dependency chain.
- LN1 output never materialized: LN2(h3 + silu(m)/1) uses LN's invariance
  to per-row affine maps: LN2(rs*(r1 - mu) + u) == LN2(r1 + u/rs), and
  u/rs = q / (1 + exp(-rs*q)) with q = r1@W_mlp - mu(x)cs in psum.
- residual r1 = h + attn@W_out accumulated in PSUM via an identity-matmul
  of h (no DVE add)
- rstd = exp(-0.5*ln(var+eps)) so every ACT func (exp/ln/copy/identity)
  lives in ONE table (natural_log_exp_and_others, id 6) -> zero
  ACT_TABLE_LOADs in steady state (explicitly loaded once up front)
- denominators: exp -> bf16 E, DVE reduce, DVE reciprocal; V scaled by
  1/D directly from psum (no Vn sbuf copy)

Sharding: 8 systems (512 electrons) per NeuronCore, parameters replicated.
"""

import sys

if "/opt/trn_rl_repo" not in sys.path:
    sys.path.insert(0, "/opt/trn_rl_repo")

from contextlib import ExitStack

import numpy as np

N_SYS = 64
N_ELEC = 64
DIM = 256
HEADS = 8
HD = DIM // HEADS  # 32
EPS = 1e-6
NCORES = 8
SPC = N_SYS // NCORES      # systems per core = 8
R = SPC * N_ELEC           # rows per core = 512
NPAIR = SPC // 2           # 128-row pairs per core = 4
NBLK = R // 128            # 128-row blocks per core = 4
SCALE = 1.0 / float(np.sqrt(HD))
ACT_TABLE_ID = 6           # natural_log_exp_and_others
N_WARM = 6                 # PE pstate-warming matmuls per chain iteration

_BUILD_CACHE: dict = {}


def _expected_edges():
    ii, jj = np.meshgrid(np.arange(N_ELEC), np.arange(N_ELEC), indexing="ij")
    offs = (np.arange(N_SYS) * N_ELEC)[:, None, None]
    ei = (offs + ii[None]).reshape(-1).astype(np.int32)
    ej = (offs + jj[None]).reshape(-1).astype(np.int32)
    return ei, ej


def _edges_are_blockdense(e_e_i, e_e_j):
    ei, ej = _expected_edges()
    a = np.asarray(e_e_i).ravel()
    b = np.asarray(e_e_j).ravel()
    if a.shape != ei.shape or b.shape != ej.shape:
        return False
    if np.array_equal(a, ei) and np.array_equal(b, ej):
        return True
    key = a.astype(np.int64) * (N_SYS * N_ELEC) + b.astype(np.int64)
    kref = ei.astype(np.int64) * (N_SYS * N_ELEC) + ej.astype(np.int64)
    return np.array_equal(np.sort(key), np.sort(kref))


def _reference_np(h_one, W_qkv, W_out, ln1_scale, ln1_bias, W_mlp, b_mlp,
                  ln2_scale, ln2_bias, e_e_i, e_e_j):
    """Numpy fallback for arbitrary edge lists (never hit for real inputs)."""
    h = np.asarray(h_one, np.float64)
    n = h.shape[0]
    qkv = h @ np.asarray(W_qkv, np.float64)
    Q, K, V = np.split(qkv, 3, axis=-1)
    Q = Q.reshape(n, HEADS, HD)
    K = K.reshape(n, HEADS, HD)
    V = V.reshape(n, HEADS, HD)
    ei = np.asarray(e_e_i).ravel()
    ej = np.asarray(e_e_j).ravel()
    A = np.einsum("ehd,ehd->eh", Q[ei], K[ej]) / np.sqrt(HD)
    mx = np.full((n, HEADS), -np.inf)
    np.maximum.at(mx, ej, A)
    e = np.exp(A - mx[ej])
    den = np.zeros((n, HEADS))
    np.add.at(den, ej, e)
    P = e / den[ej]
    attn = np.zeros((n, HEADS, HD))
    np.add.at(attn, ei, P[..., None] * V[ej])
    attn = attn.reshape(n, DIM)
    hh = h + attn @ np.asarray(W_out, np.float64)

    def ln(x, s, b):
        mu = x.mean(-1, keepdims=True)
        var = ((x - mu) ** 2).mean(-1, keepdims=True)
        return (x - mu) / np.sqrt(var + EPS) * np.asarray(s, np.float64) \
            + np.asarray(b, np.float64)

    hh = ln(hh, ln1_scale, ln1_bias)
    m = hh @ np.asarray(W_mlp, np.float64) + np.asarray(b_mlp, np.float64)
    hh = hh + m / (1.0 + np.exp(-m))
    hh = ln(hh, ln2_scale, ln2_bias)
    return hh.astype(np.float32)


def _build(flags, chain=1):
    """Build + compile the Bass program.

    flags = (ln1_aff, ln2_aff, has_cb). chain>1 repeats the body, iteration
    t reading h from the (bf16) out tensor written by t-1.
    """
    import os
    stage = os.environ.get("K2_STAGE", "full")
    key = (flags, chain, stage)
    if key in _BUILD_CACHE:
        return _BUILD_CACHE[key]

    import concourse.bass as bass
    import concourse.mybir as mybir
    import concourse.tile as tile
    from concourse import bacc
    from concourse.masks import make_identity

    ln1_aff, ln2_aff, has_cb = flags
    f32 = mybir.dt.float32
    bf16 = mybir.dt.bfloat16
    PS = bass.MemorySpace.PSUM

    nc = bacc.Bacc("TRN2", target_bir_lowering=False, debug=False,
                   num_devices=NCORES)

    h_d = nc.dram_tensor("h", [R, DIM], bf16, kind="ExternalInput")
    # packed weights: per k-chunk cols = [wq 768 | wo 256 | wm 256 | wom 256
    # | ncs-replicated 256] (ncs only meaningful in k=0)
    wp_d = nc.dram_tensor("wp", [128, 2, 1792], bf16, kind="ExternalInput")
    if has_cb:
        cb_d = nc.dram_tensor("cb", [DIM], f32, kind="ExternalInput")
    if ln1_aff:
        g1_d = nc.dram_tensor("g1", [DIM], f32, kind="ExternalInput")
        b1_d = nc.dram_tensor("b1", [DIM], f32, kind="ExternalInput")
    if ln2_aff:
        g2_d = nc.dram_tensor("g2", [DIM], f32, kind="ExternalInput")
        b2_d = nc.dram_tensor("b2", [DIM], f32, kind="ExternalInput")
    out_d = nc.dram_tensor("out", [R, DIM], bf16, kind="ExternalOutput")

    Exp = mybir.ActivationFunctionType.Exp
    Ln = mybir.ActivationFunctionType.Ln
    SUB = mybir.AluOpType.subtract
    MUL = mybir.AluOpType.mult
    ADD = mybir.AluOpType.add
    DIV = mybir.AluOpType.divide
    X = mybir.AxisListType.X

    with tile.TileContext(nc) as tc:
        with (
            tc.tile_pool(name="per", bufs=1) as per,
            tc.tile_pool(name="rot", bufs=3) as rot,
            tc.tile_pool(name="rotE", bufs=2) as rotE,
            tc.tile_pool(name="rotVp", bufs=2) as rotVp,
            tc.tile_pool(name="rotW", bufs=4) as rotW,
            tc.tile_pool(name="small", bufs=4) as small,
        ):
            # ---- persistent SBUF ----
            ident = per.tile([128, 128], f32, tag="ident")
            make_identity(nc, ident)
            identb = per.tile([128, 128], bf16, tag="identb")
            nc.vector.tensor_copy(out=identb, in_=ident)
            epst = per.tile([128, 1], f32, tag="epst")
            nc.vector.memset(epst, EPS)
            zt = per.tile([128, 1], f32, tag="zt")
            nc.vector.memset(zt, 0.0)
            onesb = per.tile([1, 128], bf16, tag="onesb")
            nc.vector.memset(onesb, 1.0)
            wall = per.tile([128, 2, 1792], bf16, tag="wall")
            wq = wall[:, :, 0:768]
            wo = wall[:, :, 768:1024]
            wm = wall[:, :, 1024:1280]
            wom = wall[:, :, 1280:1536]
            ncsB = wall[:, 0, 1536:1792]
            wuin = per.tile([128, 512], f32, tag="wuin")
            nc.vector.memset(wuin, 0.0)
            i256 = per.tile([128, 2, DIM], bf16, tag="i256")
            nc.vector.memset(i256, 0.0)
            nc.vector.tensor_copy(out=i256[:, 0, 0:128], in_=identb)
            nc.scalar.copy(out=i256[:, 1, 128:256], in_=identb)
            if has_cb:
                cbb = per.tile([128, DIM], f32, tag="cbb")
                cbr = per.tile([1, DIM], bf16, tag="cbr")
            if ln1_aff:
                g1t = per.tile([128, DIM], f32, tag="g1t")
                b1t = per.tile([128, DIM], f32, tag="b1t")
            if ln2_aff:
                g2t = per.tile([128, DIM], f32, tag="g2t")
                b2t = per.tile([128, DIM], f32, tag="b2t")
            hT = per.tile([128, 2, R], bf16, tag="hT")
            Vn = per.tile([128, NPAIR, DIM], bf16, tag="Vn")
            QT = per.tile([128, 2, R], bf16, tag="QT")
            KT = per.tile([128, 2, R], bf16, tag="KT")
            aT = per.tile([128, 2, R], bf16, tag="aT")
            mv1 = per.tile([128, NBLK, 2], f32, tag="mv1")
            mv2All = per.tile([128, NBLK, 2], f32, tag="mv2All")
            rsA = per.tile([128, NBLK], f32, tag="rsA")
            nrsA = per.tile([128, NBLK], f32, tag="nrsA")
            rs2A = per.tile([128, NBLK], f32, tag="rs2A")
            lvA = per.tile([128, NBLK], f32, tag="lvA")
            lv2A = per.tile([128, NBLK], f32, tag="lv2A")
            muTB = per.tile([128, 512], bf16, tag="muTB")
            otAll = per.tile([128, NBLK, DIM], bf16, tag="otAll")

            for it in range(chain):
                h_src = h_d if it == 0 else out_d

                nc.sync.dma_start(out=wall, in_=wp_d[:, :, :])
                for c in range(2):
                    nc.sync.dma_start_transpose(
                        out=hT[:, c, :], in_=h_src[:, 128 * c:128 * (c + 1)])
                if it > 0 and N_WARM > 0:
                    with tc.tile_pool(name=f"pswu{it}", bufs=1,
                                      space=PS) as pswu:
                        wu = pswu.tile([128, 512], f32, tag="wu",
                                       name=f"wu{it}")
                        for widx in range(N_WARM):
                            nc.tensor.matmul(wu, ident, wuin,
                                             start=(widx == 0),
                                             stop=(widx == N_WARM - 1))
                if has_cb:
                    nc.sync.dma_start(out=cbb, in_=cb_d[:].rearrange("(a d) -> a d", a=1).to_broadcast([128, DIM]))
                    nc.sync.dma_start(out=cbr, in_=cb_d[:].rearrange("(a d) -> a d", a=1))
                if ln1_aff:
                    nc.sync.dma_start(out=g1t, in_=g1_d[:].rearrange("(a d) -> a d", a=1).to_broadcast([128, DIM]))
                    nc.sync.dma_start(out=b1t, in_=b1_d[:].rearrange("(a d) -> a d", a=1).to_broadcast([128, DIM]))
                if ln2_aff:
                    nc.sync.dma_start(out=g2t, in_=g2_d[:].rearrange("(a d) -> a d", a=1).to_broadcast([128, DIM]))
                    nc.sync.dma_start(out=b2t, in_=b2_d[:].rearrange("(a d) -> a d", a=1).to_broadcast([128, DIM]))

                # ---- phase A: QKV projections + h@W_mlp ----
                it_ctx = ExitStack()
                psm = it_ctx.enter_context(
                    tc.tile_pool(name=f"psm{it}", bufs=1, space=PS))
                pa = ExitStack()
                psvp = pa.enter_context(
                    tc.tile_pool(name=f"psv{it}", bufs=1, space=PS))
                psqk = pa.enter_context(
                    tc.tile_pool(name=f"psqk{it}", bufs=2, space=PS))
                vtiles = [psvp.tile([128, 2, DIM], f32, tag=f"vp{i}",
                                    name=f"vt{it}_{i}") for i in range(2)]
                vps = [vtiles[q // 2][:, q % 2, :] for q in range(NPAIR)]
                mtiles = [psm.tile([128, 2, DIM], f32, tag=f"pm{i}",
                                   name=f"mt{it}_{i}") for i in range(2)]
                psms = [mtiles[n // 2][:, n % 2, :] for n in range(NBLK)]

                qkps = [psqk.tile([128, R], f32, tag=f"psqk{t % 2}",
                                  name=f"qkps{it}_{t}") for t in range(4)]
                for t in range(4):
                    for k in range(2):
                        nc.tensor.matmul(
                            qkps[t],
                            wq[:, k, 128 * t:128 * (t + 1)],
                            hT[:, k, :],
                            start=(k == 0), stop=(k == 1),
                        )
                    # half-0 copy right away so group-0 scores can start;
                    # half-1 copies issued after all half-0s
                    dst = QT if t < 2 else KT
                    cols0 = slice(0, 256)
                    if t % 2 == 0:
                        nc.vector.tensor_copy(out=dst[:, t % 2, cols0],
                                              in_=qkps[t][:, cols0])
                    else:
                        nc.scalar.copy(out=dst[:, t % 2, cols0],
                                       in_=qkps[t][:, cols0])
                for t in range(4):
                    dst = QT if t < 2 else KT
                    cols1 = slice(256, 512)
                    if t % 2 == 0:
                        nc.vector.tensor_copy(out=dst[:, t % 2, cols1],
                                              in_=qkps[t][:, cols1])
                    else:
                        nc.scalar.copy(out=dst[:, t % 2, cols1],
                                       in_=qkps[t][:, cols1])
                for q in range(NPAIR):
                    for k in range(2):
                        nc.tensor.matmul(
                            vps[q],
                            hT[:, k, 128 * q:128 * (q + 1)],
                            wq[:, k, 2 * DIM:3 * DIM],
                            start=(k == 0 and q % 2 == 0), stop=(k == 1),
                            skip_group_check=True,
                        )
                    if q % 2 == 0:
                        nc.vector.tensor_copy(out=Vn[:, q, :], in_=vps[q])
                    else:
                        nc.scalar.copy(out=Vn[:, q, :], in_=vps[q])
                for n in range(NBLK):
                    for k in range(2):
                        nc.tensor.matmul(
                            psms[n],
                            hT[:, k, 128 * n:128 * (n + 1)],
                            wm[:, k, :],
                            # start only on the bank's first tenant: start
                            # marks the WHOLE 2KB zero-region pending, so the
                            # second block rides the same pending flag
                            start=(k == 0 and n % 2 == 0), stop=(k == 1),
                            skip_group_check=True,
                        )
                pa.close()
                del vps

                if stage == "A":
                    nc.vector.tensor_copy(out=otAll[:, 0, :],
                                          in_=QT[:, 0, 0:256])
                    nc.vector.tensor_copy(out=otAll[:, 1, :],
                                          in_=KT[:, 0, 0:256])
                    nc.vector.tensor_copy(out=otAll[:, 2, :],
                                          in_=hT[:, 0, 0:256])
                    nc.scalar.copy(out=otAll[:, 3, :], in_=hT[:, 1, 0:256])
                    nc.sync.dma_start(
                        out=out_d[:].rearrange("(n p) d -> p n d", p=128),
                        in_=otAll)
                    it_ctx.close()
                    continue
                # ---- phase B: attention ----
                # sub-tile (tile_position) matmul outputs must sit in a psum
                # bank's first 1KB, so every tp-mm target gets its own
                # bank-rounded [128,256] tile: at_ps x4 persistent, sp x2
                # rotating (scores emitted per hh-pair sub-round)
                pb = ExitStack()
                pat = pb.enter_context(
                    tc.tile_pool(name=f"pat{it}", bufs=1, space=PS))
                psS = pb.enter_context(
                    tc.tile_pool(name=f"psS{it}", bufs=1, space=PS))
                at_ps = [[pat.tile([128, NPAIR * 64], f32, tag=f"at{c}{p2}",
                                   name=f"at2_{it}_{c}{p2}")
                          for p2 in range(2)] for c in range(2)]

                def emit_scores_hp(g, hp, sp2):
                    for hh in (2 * hp, 2 * hp + 1):
                        for p2 in range(2):
                            q = 2 * g + p2
                            for ch in range(2):
                                for par in range(2):
                                    col = 64 * (2 * q + par)
                                    nc.tensor.matmul(
                                        sp2[hh % 2][64 * par:64 * (par + 1),
                                                    128 * p2 + 64 * ch:
                                                    128 * p2 + 64 * (ch + 1)],
                                        KT[:, ch, :][32 * hh:32 * (hh + 1),
                                                     col:col + 64],
                                        QT[:, ch, :][32 * hh:32 * (hh + 1),
                                                     col:col + 64],
                                        tile_position=(32 * hh, 64 * par),
                                        start=True, stop=True,
                                    )

                def emit_exp_red(g, hp, sp2, E, Dn):
                    Eg = E[:].rearrange("p (s c h i) -> p s c h i",
                                        s=2, c=2, i=64)
                    Dg = Dn[:].rearrange("p (s c h) -> p s c h", s=2, c=2)
                    for s in range(2):
                        for hh in (2 * hp, 2 * hp + 1):
                            nc.scalar.activation(
                                out=Eg[:, s:s + 1, :, hh:hh + 1, :],
                                in_=sp2[hh % 2][:, 128 * s:128 * (s + 1)]
                                .rearrange("p (a c b i) -> p a c b i",
                                           a=1, b=1, i=64),
                                func=Exp, bias=zt, scale=SCALE,
                            )
                        nc.vector.reduce_sum(
                            out=Dg[:, s:s + 1, :, 2 * hp:2 * hp + 2],
                            in_=Eg[:, s:s + 1, :, 2 * hp:2 * hp + 2, :],
                            axis=X)

                def emit_pv(g, E, Rc):
                    Vps = []
                    for p2 in range(2):
                        q = 2 * g + p2
                        Vp = rotVp.tile([128, DIM], bf16, tag="Vp",
                                        name=f"Vp{it}_{g}_{p2}")
                        nc.vector.tensor_mul(
                            Vp[:].rearrange("p (h d) -> p h d", d=HD),
                            Vn[:, q, :].rearrange("p (h d) -> p h d", d=HD),
                            Rc[:, 8 * p2:8 * (p2 + 1)].to_broadcast(
                                [128, 8, HD]),
                        )
                        Vps.append(Vp)
                    for ch in range(2):
                        for p2 in range(2):
                            q = 2 * g + p2
                            Vp = Vps[p2]
                            for hh in range(4):
                                    hg = 4 * ch + hh
                                    for par in range(2):
                                        nc.tensor.matmul(
                                            at_ps[ch][par][
                                                32 * hh:32 * (hh + 1),
                                                64 * q:64 * (q + 1)],
                                            Vp[64 * par:64 * (par + 1),
                                               32 * hg:32 * (hg + 1)],
                                            E[64 * par:64 * (par + 1),
                                              512 * p2 + 64 * hg:
                                              512 * p2 + 64 * (hg + 1)],
                                            tile_position=(64 * par, 32 * hh),
                                            start=True, stop=True,
                                        )

                EDs = []
                for g in range(2):
                    E = rotE.tile([128, 2 * 512], bf16, tag="E",
                                  name=f"E{it}_{g}")
                    Dn = small.tile([128, 16], f32, tag="Dn",
                                    name=f"Dn{it}_{g}")
                    for hp in range(2):
                        sp2 = [psS.tile([128, 256], f32, tag=f"sp{i}",
                                        name=f"sp{it}_{g}_{hp}_{i}")
                               for i in range(2)]
                        emit_scores_hp(g, hp, sp2)
                        emit_exp_red(g, hp, sp2, E, Dn)
                    Rc = small.tile([128, 16], f32, tag="Rc",
                                    name=f"Rc{it}_{g}")
                    nc.vector.reciprocal(out=Rc, in_=Dn)
                    EDs.append((E, Rc))
                    if g == 1:
                        emit_pv(0, *EDs[0])
                emit_pv(1, *EDs[1])
                E0, E1 = EDs[0][0], EDs[1][0]

                # attnT psum -> sbuf bf16 (interleave parities)
                for c in range(2):
                    av = aT[:, c, :].rearrange("p (q s e) -> p q s e", s=2, e=64)
                    nc.vector.tensor_copy(
                        out=av[:, :, 0, :],
                        in_=at_ps[c][0][:].rearrange("p (q e) -> p q e", e=64))
                    nc.scalar.copy(
                        out=av[:, :, 1, :],
                        in_=at_ps[c][1][:].rearrange("p (q e) -> p q e", e=64))

                pb.close()

                if stage == "B":
                    nc.vector.tensor_copy(out=otAll[:, 0, :],
                                          in_=aT[:, 0, 0:256])
                    nc.vector.tensor_copy(out=otAll[:, 1, :],
                                          in_=aT[:, 1, 0:256])
                    nc.vector.tensor_copy(out=otAll[:, 2, :],
                                          in_=E0[:, 0:256])
                    nc.scalar.copy(out=otAll[:, 3, :], in_=E1[:, 0:256])
                    nc.sync.dma_start(
                        out=out_d[:].rearrange("(n p) d -> p n d", p=128),
                        in_=otAll)
                    it_ctx.close()
                    continue
                # ---- phase C ----
                pc = ExitStack()
                psh2 = pc.enter_context(
                    tc.tile_pool(name=f"psh2{it}", bufs=2, space=PS))
                psmu = pc.enter_context(
                    tc.tile_pool(name=f"psmu{it}", bufs=1, space=PS))

                # per-block pipeline, no cross-block barriers:
                # ps2 = attn@W_out + h (psum, h via identity-chunk matmuls of
                # hT) -> LN1 stats -> per-block rstd (ln/exp) + mu^T (1-col PE
                # transpose) -> rank-1 mean fix into psm -> silu-exp chain on
                # DVE -> LN2 stats -> per-block rstd2 -> ot -> packed store
                p2tiles = [psh2.tile([128, 2, DIM], f32, tag=f"p2{i}",
                                     name=f"p2t{it}_{i}") for i in range(2)]
                mups = psmu.tile([128, 512], f32, tag="mups", name=f"mups{it}")
                ps2s = []
                for n in range(NBLK):
                    ps2 = p2tiles[n // 2][:, n % 2, :]
                    ps2s.append(ps2)
                    for c in range(2):
                        nc.tensor.matmul(
                            ps2,
                            aT[:, c, 128 * n:128 * (n + 1)],
                            wo[:, c, :],
                            start=(c == 0), stop=False,
                        )
                    for k in range(2):
                        nc.tensor.matmul(
                            ps2,
                            hT[:, k, 128 * n:128 * (n + 1)],
                            i256[:, k, :],
                            start=False, stop=(k == 1),
                        )
                    for c in range(2):
                        nc.tensor.matmul(
                            psms[n],
                            aT[:, c, 128 * n:128 * (n + 1)],
                            wom[:, c, :],
                            start=False, stop=False,
                            skip_group_check=True,
                        )
                    st = small.tile([128, 6], f32, tag="st")
                    nc.vector.bn_stats(out=st, in_=ps2)
                    nc.vector.bn_aggr(out=mv1[:, n, :], in_=st)
                    nc.scalar.activation(out=lvA[:, n:n + 1],
                                         in_=mv1[:, n, 1:2], func=Ln,
                                         bias=epst)
                    nc.scalar.activation(out=rsA[:, n:n + 1],
                                         in_=lvA[:, n:n + 1], func=Exp,
                                         bias=zt, scale=-0.5)
                    nc.vector.tensor_scalar_mul(nrsA[:, n:n + 1],
                                                rsA[:, n:n + 1], -1.0)
                    nc.tensor.transpose(mups[0:1, 128 * n:128 * (n + 1)],
                                        mv1[:, n, 0:1], ident)
                    nc.scalar.copy(
                        out=muTB[0:1, 128 * n:128 * (n + 1)],
                        in_=mups[0:1, 128 * n:128 * (n + 1)])

                ws = []
                for n in range(NBLK):
                    nc.tensor.matmul(psms[n],
                                     muTB[0:1, 128 * n:128 * (n + 1)],
                                     ncsB[0:1, :],
                                     start=False, stop=not has_cb,
                                     skip_group_check=True)
                    if has_cb:
                        nc.tensor.matmul(psms[n], onesb, cbr,
                                         start=False, stop=True,
                                         skip_group_check=True)
                    et = rot.tile([128, DIM], f32, tag="et")
                    w = rotW.tile([128, DIM], f32, tag="w")
                    if not ln1_aff and not has_cb:
                        # u' = q*sigmoid(rs*q); sigmoid via exp/ln only
                        # (keeps the single ACT table): sg = exp(-ln(1+e))
                        nc.scalar.activation(out=et, in_=psms[n], func=Exp,
                                             bias=zt, scale=nrsA[:, n:n + 1])
                        sp = rot.tile([128, DIM], f32, tag="sp")
                        nc.scalar.activation(out=sp, in_=et, func=Ln,
                                             bias=1.0)
                        sg = rot.tile([128, DIM], f32, tag="sg")
                        nc.scalar.activation(out=sg, in_=sp, func=Exp,
                                             bias=zt, scale=-1.0)
                        ut = rot.tile([128, DIM], f32, tag="ut")
                        nc.vector.scalar_tensor_tensor(
                            ut, psms[n], 1.0, sg, op0=MUL, op1=MUL)
                        nc.vector.scalar_tensor_tensor(
                            w, ps2s[n], 1.0, ut, op0=MUL, op1=ADD)
                    else:
                        # general path: explicit h3/m
                        r1 = rot.tile([128, DIM], f32, tag="r1")
                        nc.scalar.copy(out=r1, in_=ps2s[n])
                        h3 = rotW.tile([128, DIM], f32, tag="h3")
                        nc.gpsimd.tensor_scalar(h3, r1, mv1[:, n, 0:1],
                                                rsA[:, n:n + 1],
                                                op0=SUB, op1=MUL)
                        if ln1_aff:
                            nc.vector.tensor_mul(h3, h3, g1t)
                            nc.vector.tensor_add(h3, h3, b1t)
                        m = rot.tile([128, DIM], f32, tag="m")
                        nc.vector.tensor_scalar(m, psms[n], rsA[:, n:n + 1],
                                                None, op0=MUL)
                        if has_cb:
                            nc.vector.tensor_add(m, m, cbb)
                        nc.scalar.activation(out=et, in_=m, func=Exp,
                                             bias=zt, scale=-1.0)
                        sp = rot.tile([128, DIM], f32, tag="sp")
                        nc.scalar.activation(out=sp, in_=et, func=Ln,
                                             bias=1.0)
                        sg = rot.tile([128, DIM], f32, tag="sg")
                        nc.scalar.activation(out=sg, in_=sp, func=Exp,
                                             bias=zt, scale=-1.0)
                        nc.vector.tensor_mul(w, m, sg)
                        nc.gpsimd.tensor_add(w, h3, w)
                    ws.append(w)
                    st2 = small.tile([128, 6], f32, tag="st2")
                    nc.vector.bn_stats(out=st2, in_=w)
                    nc.vector.bn_aggr(out=mv2All[:, n, :], in_=st2)
                    nc.scalar.activation(out=lv2A[:, n:n + 1],
                                         in_=mv2All[:, n, 1:2], func=Ln,
                                         bias=epst)
                    nc.scalar.activation(out=rs2A[:, n:n + 1],
                                         in_=lv2A[:, n:n + 1], func=Exp,
                                         bias=zt, scale=-0.5)
                    nc.gpsimd.tensor_scalar(otAll[:, n, :], w,
                                            mv2All[:, n, 0:1],
                                            rs2A[:, n:n + 1],
                                            op0=SUB, op1=MUL)
                    if ln2_aff:
                        nc.vector.tensor_mul(otAll[:, n, :], otAll[:, n, :],
                                             g2t)
                        nc.vector.tensor_add(otAll[:, n, :], otAll[:, n, :],
                                             b2t)
                for hlf in range(2):
                    nc.sync.dma_start(
                        out=out_d[256 * hlf:256 * (hlf + 1), :].rearrange(
                            "(n p) d -> p n d", p=128),
                        in_=otAll[:, 2 * hlf:2 * (hlf + 1), :])

                pc.close()
                it_ctx.close()

    # single explicit ACT table load (set 6 covers exp/ln/copy/identity)
    import os
    if not os.environ.get("K2_NO_TABLE_PATCH"):
        _patch_single_table_load(nc)
    nc.compile()
    _BUILD_CACHE[key] = nc
    return nc


def _patch_single_table_load(nc):
    """Replace bacc's greedy per-func table insertion with one up-front
    InstLoadActFuncSet of natural_log_exp_and_others."""
    import concourse.mybir as mybir

    def _single(self=nc):
        blocks = nc.main_func.blocks
        for blk in blocks:
            for idx, inst in enumerate(blk.instructions):
                if getattr(inst, "engine", None) == mybir.EngineType.Activation:
                    load = mybir.InstLoadActFuncSet(
                        name=nc.get_next_instruction_name(), ins=[], outs=[])
                    load.act_func_set_id = ACT_TABLE_ID
                    load.engine = mybir.EngineType.Activation
                    nc.register_instruction(load)
                    blk.instructions.insert(idx, load)
                    return

    nc.insert_act_table_loads = _single


def _prep_weights(W_qkv, W_out, ln1_scale, W_mlp, b_mlp, ln1_bias):
    bf16 = np.dtype("bfloat16") if hasattr(np, "bfloat16") else None
    import ml_dtypes
    bf = ml_dtypes.bfloat16
    Wm_eff = (np.asarray(ln1_scale, np.float64)[:, None]
              * np.asarray(W_mlp, np.float64))
    Wom = np.asarray(W_out, np.float64) @ Wm_eff
    ncs = -Wm_eff.sum(axis=0)
    cb = (np.asarray(ln1_bias, np.float64) @ np.asarray(W_mlp, np.float64)
          + np.asarray(b_mlp, np.float64))
    return (
        np.asarray(W_qkv, np.float32).astype(bf),
        np.asarray(W_out, np.float32).astype(bf),
        Wm_eff.astype(np.float32).astype(bf),
        Wom.astype(np.float32).astype(bf),
        ncs.astype(np.float32).astype(bf),
        cb.astype(np.float32),
    )


def kernel(h_one, W_qkv, W_out, ln1_scale, ln1_bias, W_mlp, b_mlp,
           ln2_scale, ln2_bias, e_e_i, e_e_j, _trace=False, _chain=1):
    import ml_dtypes
    bf = ml_dtypes.bfloat16

    h_one = np.ascontiguousarray(np.asarray(h_one, np.float32))
    ln1_scale = np.asarray(ln1_scale, np.float32)
    ln1_bias = np.asarray(ln1_bias, np.float32)
    ln2_scale = np.asarray(ln2_scale, np.float32)
    ln2_bias = np.asarray(ln2_bias, np.float32)
    b_mlp = np.asarray(b_mlp, np.float32)

    if not _edges_are_blockdense(e_e_i, e_e_j):
        return _reference_np(h_one, W_qkv, W_out, ln1_scale, ln1_bias, W_mlp,
                             b_mlp, ln2_scale, ln2_bias, e_e_i, e_e_j)

    wqb, wob, wmb, womb, ncsb, cb = _prep_weights(
        W_qkv, W_out, ln1_scale, W_mlp, b_mlp, ln1_bias)
    ln1_aff = not (np.all(ln1_scale == 1.0) and np.all(ln1_bias == 0.0))
    ln2_aff = not (np.all(ln2_scale == 1.0) and np.all(ln2_bias == 0.0))
    has_cb = bool(np.any(cb != 0.0))
    flags = (ln1_aff, ln2_aff, has_cb)
    nc = _build(flags, chain=_chain)

    from concourse.bass_utils import run_bass_kernel_spmd

    hb = h_one.astype(bf)
    wpack = np.zeros((128, 2, 1792), dtype=bf)
    for k in range(2):
        rows = slice(128 * k, 128 * (k + 1))
        wpack[:, k, 0:768] = wqb[rows, :]
        wpack[:, k, 768:1024] = wob[rows, :]
        wpack[:, k, 1024:1280] = wmb[rows, :]
        wpack[:, k, 1280:1536] = womb[rows, :]
    wpack[:, 0, 1536:1792] = ncsb[None, :]
    in_maps = []
    for c in range(NCORES):
        m = {
            "h": hb[R * c:R * (c + 1)],
            "wp": wpack,
        }
        if has_cb:
            m["cb"] = cb
        if ln1_aff:
            m["g1"] = ln1_scale
            m["b1"] = ln1_bias
        if ln2_aff:
            m["g2"] = ln2_scale
            m["b2"] = ln2_bias
        in_maps.append(m)

    try:
        res = run_bass_kernel_spmd(nc, in_maps, core_ids=list(range(NCORES)),
                                   trace=_trace)
    except ModuleNotFoundError:
        res = run_bass_kernel_spmd(nc, in_maps, core_ids=list(range(NCORES)),
                                   trace=False)
    out = np.concatenate(
        [np.asarray(res.results[c]["out"]).astype(np.float32)
         for c in range(NCORES)], axis=0)
    if _trace:
        kernel._last_results = res
    return out
